# revision 6
# baseline (speedup 1.0000x reference)
"""Trainium2 Bass kernel for nn_BioinspiredNeuralNetwork (3-layer holographic MLP).

Math per layer i (complex):
    out = xc @ (Wr + i*Wi)
    act = sigmoid(beta_i * out.real) ** alpha_i
    xc  = act * out / |out| * mask_i     (mask: fixed PRNG key 42, host-precomputed)

Distribution: tensor-parallel over output columns across 8 cores (512 cols
each). Activations kept transposed [4096, 512] (neuron-major), replicated via
AllGather at each layer boundary.

Precision: 3-pass fp16 split matmuls (x = x_hi + x_lo, W = W_hi + W_lo, drop
lo*lo) — fp32-class output error at full 1-cycle/row PE rate. Verified on HW:
rms rel err 2.6e-7 per 4096-deep matmul.

Overlap: each layer computes its 4 output m-tiles in two phases (m01, m23).
The boundary AllGather is split in two halves; AG half A (rows 0:256 of every
shard) launches after phase 1's epilogue and hides under phase 2's matmuls;
AG half B hides under the next layer's first-half chunks. k-chunk loops run
A-chunks first so boundary reloads can overwrite them early (WAR pipelining).
Complex real part uses two PSUM banks (P1 = x_r@W_r, P2 = x_i@W_i) combined
as P1 - P2 in the epilogue — no negated weights needed.
"""

import numpy as np

import concourse.bass as bass
import concourse.mybir as mybir
import concourse.tile as tile
from concourse import bacc
from concourse.bass import ds, ts
from concourse.bass_utils import run_bass_kernel_spmd

AF = mybir.ActivationFunctionType

NCORES = 8
B = 512            # batch
D = 4096           # layer width
S = D // NCORES    # per-core output column shard (512)
KT = D // 128      # 32 k-chunks
MT = S // 128      # 4 m-tiles per shard
N_LAYERS = 3
N_CLUSTERS = 10
SPARSITY = 0.2

f32 = mybir.dt.float32
f16 = mybir.dt.float16

# chunk order: AllGather half A covers rows 0:256 of every core's shard
# (chunks 4c, 4c+1), half B the rest. Every k-loop processes A-chunks first.
_CHUNKS_A = [4 * c + j for c in range(NCORES) for j in (0, 1)]
_CHUNKS_B = [4 * c + j for c in range(NCORES) for j in (2, 3)]
_CHUNKS = _CHUNKS_A + _CHUNKS_B


def _build(betas, alphas):
    nc = bacc.Bacc("TRN2", target_bir_lowering=False, debug=False,
                   num_devices=NCORES)

    xt = nc.dram_tensor("xt", [D, B], f32, kind="ExternalInput")
    # weight piece packs: [D, 2*S] fp16, cols 0:S = hi piece, S:2S = lo piece
    wrp = [nc.dram_tensor(f"w{l}rp", [D, 2 * S], f16, kind="ExternalInput")
           for l in range(N_LAYERS)]
    wip = [nc.dram_tensor(f"w{l}ip", [D, 2 * S], f16, kind="ExternalInput")
           for l in range(N_LAYERS)]
    msk = [nc.dram_tensor(f"mask{l}", [S, B], f32, kind="ExternalInput")
           for l in range(N_LAYERS)]
    outr = nc.dram_tensor("outr", [S, B], f32, kind="ExternalOutput")
    outi = nc.dram_tensor("outi", [S, B], f32, kind="ExternalOutput")

    # exchange buffers per boundary/half: rows [xr(2x128); xi(2x128)]
    agi = [[nc.dram_tensor(f"agi{b}{h}", [S, B], f32) for h in range(2)]
           for b in range(2)]
    ago = [[nc.dram_tensor(f"ago{b}{h}", [NCORES * S, B], f32,
                           addr_space="Shared") for h in range(2)]
           for b in range(2)]

    xt_r = xt.ap().rearrange("(n p) b -> n p b", p=128)
    wrp_r = [w.ap().rearrange("(n p) s -> n p s", p=128) for w in wrp]
    wip_r = [w.ap().rearrange("(n p) s -> n p s", p=128) for w in wip]
    msk_r = [m.ap().rearrange("(n p) b -> n p b", p=128) for m in msk]

    with tile.TileContext(nc) as tc:
        with (
            tc.tile_pool(name="xp", bufs=1) as xp,
            tc.tile_pool(name="wp", bufs=4) as wp,
            tc.tile_pool(name="rp", bufs=4) as rp,
            tc.tile_pool(name="mp", bufs=1) as mp,
            tc.tile_pool(name="ep", bufs=1) as ep,
            tc.tile_pool(name="ps", bufs=1, space="PSUM") as ps,
        ):
            # resident fp16 hi/lo pieces of transposed activations
            xrh = [xp.tile([128, B], f16, name=f"xrh{k}", tag=f"xrh{k}")
                   for k in range(KT)]
            xrl = [xp.tile([128, B], f16, name=f"xrl{k}", tag=f"xrl{k}")
                   for k in range(KT)]
            xih = [xp.tile([128, B], f16, name=f"xih{k}", tag=f"xih{k}")
                   for k in range(KT)]
            xil = [xp.tile([128, B], f16, name=f"xil{k}", tag=f"xil{k}")
                   for k in range(KT)]

            def load_split(k, src_ap, hi, lo, tag):
                t = rp.tile([128, B], f32, name=tag, tag=tag)
                nc.sync.dma_start(out=t[:], in_=src_ap)
                nc.vector.tensor_copy(hi[k][:], t[:])
                nc.vector.tensor_sub(lo[k][:], t[:], hi[k][:])

            for k in _CHUNKS:
                load_split(k, xt_r[k], xrh, xrl, "rlr")

            def mm_phase(l, pair, scope):
                """One m-pair phase of layer l's matmuls. Returns psum tiles."""
                with nc.named_scope(scope):
                    # phase-local PSUM tags (6 banks max, reused across phases)
                    p1 = {m: ps.tile([128, B], f32, name=f"p1s{i}", tag=f"p1s{i}")
                          for i, m in enumerate(pair)}
                    pi = {m: ps.tile([128, B], f32, name=f"pis{i}", tag=f"pis{i}")
                          for i, m in enumerate(pair)}
                    p2 = ({m: ps.tile([128, B], f32, name=f"p2s{i}",
                                      tag=f"p2s{i}") for i, m in enumerate(pair)}
                          if l > 0 else None)
                    for ki, k in enumerate(_CHUNKS):
                        first, last = ki == 0, ki == KT - 1
                        wr_t = wp.tile([128, 2 * S], f16, name="wr_t", tag="wr_t")
                        nc.sync.dma_start(out=wr_t[:], in_=wrp_r[l][k])
                        wi_t = wp.tile([128, 2 * S], f16, name="wi_t", tag="wi_t")
                        nc.sync.dma_start(out=wi_t[:], in_=wip_r[l][k])
                        for m in pair:
                            rh = wr_t[:, ts(m, 128)]            # wr_hi
                            rl = wr_t[:, ds(S + m * 128, 128)]  # wr_lo
                            ih = wi_t[:, ts(m, 128)]
                            il = wi_t[:, ds(S + m * 128, 128)]
                            mm = nc.tensor.matmul
                            # P1 = xr @ wr  (3-pass)
                            mm(p1[m][:], lhsT=rh, rhs=xrh[k][:],
                               start=first, stop=False)
                            mm(p1[m][:], lhsT=rh, rhs=xrl[k][:],
                               start=False, stop=False)
                            mm(p1[m][:], lhsT=rl, rhs=xrh[k][:],
                               start=False, stop=last)
                            # Pi += xr @ wi
                            mm(pi[m][:], lhsT=ih, rhs=xrh[k][:],
                               start=first, stop=False)
                            mm(pi[m][:], lhsT=ih, rhs=xrl[k][:],
                               start=False, stop=False)
                            mm(pi[m][:], lhsT=il, rhs=xrh[k][:],
                               start=False, stop=(last and l == 0))
                            if l > 0:
                                # P2 = xi @ wi  (subtracted in epilogue)
                                mm(p2[m][:], lhsT=ih, rhs=xih[k][:],
                                   start=first, stop=False)
                                mm(p2[m][:], lhsT=ih, rhs=xil[k][:],
                                   start=False, stop=False)
                                mm(p2[m][:], lhsT=il, rhs=xih[k][:],
                                   start=False, stop=last)
                                # Pi += xi @ wr
                                mm(pi[m][:], lhsT=rh, rhs=xih[k][:],
                                   start=False, stop=False)
                                mm(pi[m][:], lhsT=rh, rhs=xil[k][:],
                                   start=False, stop=False)
                                mm(pi[m][:], lhsT=rl, rhs=xih[k][:],
                                   start=False, stop=last)
                    return p1, p2, pi

            def epilogue(l, pair, p1, p2, pi, mt, scope):
                with nc.named_scope(scope):
                    for m in pair:
                        if p2 is not None:
                            # DVE can read only one PSUM operand per op
                            p2s = ep.tile([128, B], f32, name="p2s", tag="p2s")
                            nc.scalar.copy(p2s[:], p2[m][:])
                            orr = ep.tile([128, B], f32, name="orr", tag="orr")
                            nc.vector.tensor_sub(orr[:], p1[m][:], p2s[:])
                            orr_ap = orr[:]
                        else:
                            orr_ap = p1[m][:]
                        act = ep.tile([128, B], f32, name="act", tag="act")
                        nc.scalar.activation(act[:], orr_ap, AF.Sigmoid,
                                             scale=float(betas[l]))
                        if abs(alphas[l] - 1.0) > 1e-12:
                            lg = ep.tile([128, B], f32, name="lg", tag="lg")
                            nc.scalar.activation(lg[:], act[:], AF.Ln)
                            nc.scalar.activation(act[:], lg[:], AF.Exp,
                                                 scale=float(alphas[l]))
                        t0 = ep.tile([128, B], f32, name="t0", tag="t0")
                        nc.scalar.activation(t0[:], orr_ap, AF.Square)
                        t1 = ep.tile([128, B], f32, name="t1", tag="t1")
                        nc.scalar.activation(t1[:], pi[m][:], AF.Square)
                        s = ep.tile([128, B], f32, name="s", tag="s")
                        nc.vector.tensor_add(s[:], t0[:], t1[:])
                        rin = ep.tile([128, B], f32, name="rin", tag="rin")
                        nc.vector.reciprocal(rin[:], s[:])
                        q = ep.tile([128, B], f32, name="q", tag="q")
                        nc.scalar.activation(q[:], rin[:], AF.Sqrt)
                        f = ep.tile([128, B], f32, name="f", tag="f")
                        nc.vector.tensor_mul(f[:], act[:], q[:])
                        fm = ep.tile([128, B], f32, name="fm", tag="fm")
                        nc.vector.tensor_mul(fm[:], f[:], mt[m][:])
                        xnr = ep.tile([128, B], f32, name="xnr", tag="xnr")
                        nc.vector.tensor_mul(xnr[:], fm[:], orr_ap)
                        xni = ep.tile([128, B], f32, name="xni", tag="xni")
                        nc.vector.tensor_mul(xni[:], fm[:], pi[m][:])
                        if l == N_LAYERS - 1:
                            nc.sync.dma_start(out=outr.ap()[ts(m, 128)],
                                              in_=xnr[:])
                            nc.sync.dma_start(out=outi.ap()[ts(m, 128)],
                                              in_=xni[:])
                        else:
                            h, j = divmod(m, 2)
                            dst = agi[l][h].ap()
                            nc.sync.dma_start(out=dst[ds(j * 128, 128)],
                                              in_=xnr[:])
                            nc.sync.dma_start(out=dst[ds(256 + j * 128, 128)],
                                              in_=xni[:])

            def allgather(l, h):
                nc.gpsimd.collective_compute(
                    "AllGather", mybir.AluOpType.bypass,
                    ins=[agi[l][h].ap().opt()],
                    outs=[ago[l][h].ap().opt()],
                    replica_groups=[list(range(NCORES))],
                )

            def reload_half(l, h, scope):
                with nc.named_scope(scope):
                    gao = ago[l][h].ap()
                    for c in range(NCORES):
                        for j in range(2):
                            k = 4 * c + 2 * h + j
                            base = c * S + j * 128
                            load_split(k, gao[ds(base, 128)], xrh, xrl, "rlr")
                            load_split(k, gao[ds(base + 256, 128)], xih, xil,
                                       "rli")

            for l in range(N_LAYERS):
                mt = [mp.tile([128, B], f32, name=f"mt{m}", tag=f"mt{m}")
                      for m in range(MT)]
                for m in range(MT):
                    nc.sync.dma_start(out=mt[m][:], in_=msk_r[l][m])

                p1, p2, pi = mm_phase(l, (0, 1), f"l{l}p1")
                epilogue(l, (0, 1), p1, p2, pi, mt, f"l{l}e1")
                if l < N_LAYERS - 1:
                    with nc.named_scope(f"x{l}a"):
                        allgather(l, 0)
                p1, p2, pi = mm_phase(l, (2, 3), f"l{l}p2")
                epilogue(l, (2, 3), p1, p2, pi, mt, f"l{l}e2")
                if l < N_LAYERS - 1:
                    with nc.named_scope(f"x{l}b"):
                        allgather(l, 1)
                    reload_half(l, 0, f"r{l}a")
                    reload_half(l, 1, f"r{l}b")

    nc.compile()
    return nc


_NC_CACHE: dict = {}
TRACE = False
LAST_RES = None


def _get_nc(betas, alphas):
    key = (tuple(betas), tuple(alphas))
    if key not in _NC_CACHE:
        _NC_CACHE[key] = _build(betas, alphas)
    return _NC_CACHE[key]


def _ctx_mask_host(layer_i, cw, asg, batch):
    """Exact replica of reference._ctx_mask — fixed PRNG key, depends on
    inputs only through cw (cluster weights) and asg (cluster assignment)."""
    import jax
    import jax.numpy as jnp

    cpu = jax.devices("cpu")[0]
    with jax.default_device(cpu):
        key = jax.random.fold_in(jax.random.key(42), layer_i)
        cw_j = jnp.asarray(cw)
        asg_j = jnp.asarray(asg)
        probs = jax.nn.softmax(cw_j)
        p = probs[asg_j] * SPARSITY
        n = asg.shape[0]
        k1, k2 = jax.random.split(key)
        bern = jax.random.uniform(k1, (batch, n)) < p
        u = jax.random.uniform(k2, (batch, n))
        segmax = jax.vmap(
            lambda ur: jax.ops.segment_max(ur, asg_j, num_segments=N_CLUSTERS)
        )(u)
        force = u >= segmax[:, asg_j]
        return np.asarray((bern | force).astype(jnp.float32))


def _split16(w):
    hi = w.astype(np.float16)
    lo = (w - hi.astype(np.float32)).astype(np.float16)
    return np.concatenate([hi, lo], axis=1)


def kernel(**inputs):
    x = np.asarray(inputs["x"], np.float32)
    betas = [float(v) for v in np.asarray(inputs["beta"], np.float32)]
    alphas = [float(v) for v in np.asarray(inputs["alpha"], np.float32)]

    nc = _get_nc(betas, alphas)

    xt = np.ascontiguousarray(x.T)
    masksT = [
        np.ascontiguousarray(
            _ctx_mask_host(
                l,
                np.asarray(inputs[f"cw{l}"], np.float32),
                np.asarray(inputs[f"asg{l}"]),
                x.shape[0],
            ).T
        )
        for l in range(N_LAYERS)
    ]

    in_maps = []
    for c in range(NCORES):
        sl = slice(c * S, (c + 1) * S)
        m = {"xt": xt}
        for l in range(N_LAYERS):
            m[f"w{l}rp"] = _split16(np.asarray(inputs[f"W{l}r"], np.float32)[:, sl])
            m[f"w{l}ip"] = _split16(np.asarray(inputs[f"W{l}i"], np.float32)[:, sl])
            m[f"mask{l}"] = np.ascontiguousarray(masksT[l][sl, :])
        in_maps.append(m)

    res = run_bass_kernel_spmd(nc, in_maps, core_ids=list(range(NCORES)),
                               trace=TRACE)
    global LAST_RES
    LAST_RES = res
    outr = np.concatenate([res.results[c]["outr"] for c in range(NCORES)], axis=0)
    outi = np.concatenate([res.results[c]["outi"] for c in range(NCORES)], axis=0)
    return (outr.T + 1j * outi.T).astype(np.complex64)
